# revision 3
# baseline (speedup 1.0000x reference)
# Multi-head causal self-attention (B=2, S=2048, D=768, H=12) on 8 NeuronCores.
#
# Sharding: (batch, head-group) across cores. Core c handles batch c//4 and
# heads 3*(c%4) .. 3*(c%4)+2. Each core computes its heads' Q/K/V projections
# (column-sharded), the causal attention for those heads, and a row-sharded
# partial of the output projection. Host sums the 4 partials per batch + bo.
#
# Self-contained: hardcodes shapes; builds the Bass module once per process.

import sys
import types

import numpy as np

sys.path.insert(0, "/opt/trn_rl_repo")

import concourse.bass as bass  # noqa: E402
import concourse.mybir as mybir  # noqa: E402
import concourse.tile as tile  # noqa: E402
from concourse.bass import ts  # noqa: E402
from concourse.bass_utils import run_bass_kernel_spmd  # noqa: E402

F32 = mybir.dt.float32
AF = mybir.ActivationFunctionType

B, S, D, H, HD = 2, 2048, 768, 12, 64
HPC = 3               # heads per core
DQK = 2 * HPC * HD    # 384: per-head-interleaved [Q_h | K_h] projection width
DV = HPC * HD         # 192
P = 128
IC = S // 512         # 4 query chunks of 512
KC = D // P           # 6 contraction chunks
NIO = S // P          # 16 token chunks of 128


def _split_excess_waits(nc, max_waits=1):
    # walrus in this env rejects instructions carrying more than ~1-2
    # sync-waits. Move excess waits onto preceding same-engine nops
    # (sequencer executes the nop's wait, then the instruction's).
    n_split = 0
    for func in nc.m.functions:
        for blk in func.blocks:
            insts = blk.instructions
            out = []
            changed = False
            for inst in insts:
                si = inst.sync_info
                waits = list(si.on_wait) if si and si.on_wait else []
                if len(waits) > max_waits:
                    changed = True
                    for j, w in enumerate(waits[:-max_waits]):
                        out.append(
                            mybir.InstNoOp(
                                name=f"{inst.name}-wsplit{j}",
                                engine=inst.engine,
                                ins=[],
                                outs=[],
                                sync_info=mybir.SyncInfo(
                                    on_wait=[w], on_update=[]
                                ),
                            )
                        )
                        n_split += 1
                    inst.sync_info = mybir.SyncInfo(
                        on_wait=waits[-max_waits:],
                        on_update=list(si.on_update) if si.on_update else [],
                    )
                out.append(inst)
            if changed:
                blk.instructions = out
    return n_split


def _build_module():
    nc = bass.Bass()
    xt_d = nc.dram_tensor("xt", [D, S], F32, kind="ExternalInput")
    wqk_d = nc.dram_tensor("wqk", [D, DQK], F32, kind="ExternalInput")
    bqk_d = nc.dram_tensor("bqk", [1, DQK], F32, kind="ExternalInput")
    wv_d = nc.dram_tensor("wv", [D, DV], F32, kind="ExternalInput")
    bv_d = nc.dram_tensor("bv", [1, DV], F32, kind="ExternalInput")
    wos_d = nc.dram_tensor("wos", [HD, HPC, D], F32, kind="ExternalInput")
    mask_d = nc.dram_tensor("mask", [P, 4, 512], F32, kind="ExternalInput")
    out_d = nc.dram_tensor("out", [S, D], F32, kind="ExternalOutput")

    with tile.TileContext(nc) as tc:
        with (
            tc.tile_pool(name="const", bufs=1) as cp,
            tc.tile_pool(name="xtp", bufs=2) as xtp,
            tc.tile_pool(name="exp", bufs=3) as exp_p,
            tc.tile_pool(name="small", bufs=4) as sp,
            tc.tile_pool(name="outp", bufs=2) as op,
            tc.tile_pool(name="proj", bufs=2, space="PSUM") as proj_p,
            tc.tile_pool(name="scps", bufs=2, space="PSUM") as sc_p,
            tc.tile_pool(name="avps", bufs=2, space="PSUM") as av_p,
        ):
            # ---- resident SBUF tensors ----
            wqk_sb = cp.tile([P, KC, DQK], F32)
            nc.sync.dma_start(wqk_sb, wqk_d.rearrange("(kc p) d -> p kc d", p=P))
            wv_sb = cp.tile([P, KC, DV], F32)
            nc.sync.dma_start(wv_sb, wv_d.rearrange("(kc p) d -> p kc d", p=P))
            wos_sb = cp.tile([HD, HPC, D], F32)
            nc.sync.dma_start(wos_sb, wos_d[:])
            bqk_sb = cp.tile([1, DQK], F32)
            nc.sync.dma_start(bqk_sb, bqk_d[:])
            bv_sb = cp.tile([1, DV], F32)
            nc.sync.dma_start(bv_sb, bv_d[:])
            mask_sb = cp.tile([P, 4, 512], F32)
            nc.sync.dma_start(mask_sb, mask_d[:])
            ones_sb = cp.tile([1, 512], F32)
            nc.gpsimd.memset(ones_sb, 1.0)

            qT = cp.tile([HD, HPC, S], F32)       # per-head Q^T  [d, h, i]
            klo = cp.tile([HD, HPC, S], F32)      # per-head K^T  [d, h, j]
            # V plus a ones column (col HD) for the softmax denominator
            v1 = cp.tile([P, NIO, HPC, HD + 1], F32)
            nc.gpsimd.memset(v1, 1.0)
            ctxT = cp.tile([HD, HPC, S], F32)     # unnormalized... normalized ctx^T

            for ic in range(IC):
                isl = ts(ic, 512)
                xt_t = xtp.tile([P, KC, 512], F32, tag="xt")
                nc.sync.dma_start(
                    xt_t, xt_d.rearrange("(kc p) t -> p kc t", p=P)[:, :, isl]
                )

                # ---- QK projection: chunk h of psum = [Q_h | K_h] ----
                for h in range(HPC):
                    ps = proj_p.tile([P, 512], F32, tag="proj")
                    for kc in range(KC):
                        nc.tensor.matmul(
                            ps,
                            lhsT=wqk_sb[:, kc, ts(h, P)],
                            rhs=xt_t[:, kc, :],
                            start=(kc == 0),
                            stop=False,
                        )
                    nc.tensor.matmul(
                        ps,
                        lhsT=bqk_sb[:, ts(h, P)],
                        rhs=ones_sb[:, :],
                        start=False,
                        stop=True,
                    )
                    nc.vector.tensor_copy(qT[:, h, isl], ps[0:HD, :])
                    nc.vector.tensor_copy(klo[:, h, isl], ps[HD:P, :])

                # ---- V projection (natural layout, tokens on partitions) ----
                for io4 in range(4):
                    io = ic * 4 + io4
                    ps = proj_p.tile([P, 512], F32, tag="proj")
                    psv = ps[:, :DV]
                    for kc in range(KC):
                        nc.tensor.matmul(
                            psv,
                            lhsT=xt_t[:, kc, ts(io4, P)],
                            rhs=wv_sb[:, kc, :],
                            start=(kc == 0),
                            stop=False,
                        )
                    nc.tensor.matmul(
                        psv,
                        lhsT=ones_sb[:, 0:P],
                        rhs=bv_sb[:, :],
                        start=False,
                        stop=True,
                    )
                    nc.vector.tensor_copy(
                        v1[:, io, :, 0:HD],
                        psv.rearrange("p (h e) -> p h e", e=HD),
                    )

                # ---- attention for queries in this chunk ----
                n_j = 4 * ic + 4
                for h in range(HPC):
                    avp = av_p.tile([HD + 1, 512], F32, tag="av")
                    prev = None
                    for jb in range(0, n_j, 2):
                        sc = sc_p.tile([P, 2, 512], F32, tag="sc")
                        for k in range(2):
                            jc = jb + k
                            nc.tensor.matmul(
                                sc[:, k, :],
                                lhsT=klo[:, h, ts(jc, P)],
                                rhs=qT[:, h, isl],
                                start=True,
                                stop=True,
                            )
                        ex = exp_p.tile([P, 2, 512], F32, tag="ex")
                        nc.scalar.activation(ex, sc, AF.Exp)
                        for k in range(2):
                            koff = (jb + k) - 4 * ic
                            if koff >= 0:
                                nc.vector.tensor_mul(
                                    ex[:, k, :], ex[:, k, :], mask_sb[:, koff, :]
                                )
                        if prev is not None:
                            pex, pjb = prev
                            for k in range(2):
                                jc = pjb + k
                                nc.tensor.matmul(
                                    avp,
                                    lhsT=v1[:, jc, h, :],
                                    rhs=pex[:, k, :],
                                    start=(jc == 0),
                                    stop=(jc == n_j - 1),
                                )
                        prev = (ex, jb)
                    pex, pjb = prev
                    for k in range(2):
                        jc = pjb + k
                        nc.tensor.matmul(
                            avp,
                            lhsT=v1[:, jc, h, :],
                            rhs=pex[:, k, :],
                            start=(jc == 0),
                            stop=(jc == n_j - 1),
                        )
                    # normalize: ctxT = avp[0:HD] * (1/Z) with Z broadcast
                    rec = sp.tile([1, 512], F32, tag="rec")
                    nc.vector.reciprocal(rec, avp[HD : HD + 1, :])
                    rb_ps = proj_p.tile([P, 512], F32, tag="proj")
                    nc.tensor.matmul(
                        rb_ps[0:HD, :],
                        lhsT=ones_sb[0:1, 0:HD],
                        rhs=rec[:, :],
                        start=True,
                        stop=True,
                    )
                    rb_sb = sp.tile([HD, 512], F32, tag="rb")
                    nc.vector.tensor_copy(rb_sb, rb_ps[0:HD, :])
                    nc.vector.tensor_tensor(
                        ctxT[:, h, isl],
                        avp[0:HD, :],
                        rb_sb,
                        mybir.AluOpType.mult,
                    )

                # ---- output projection for this chunk's tokens ----
                for io4 in range(4):
                    io = ic * 4 + io4
                    o_sb = op.tile([P, D], F32, tag="osb")
                    for ot, ow in ((0, 512), (1, 256)):
                        ps = proj_p.tile([P, 512], F32, tag="proj")
                        pso = ps[:, :ow]
                        for h in range(HPC):
                            nc.tensor.matmul(
                                pso,
                                lhsT=ctxT[:, h, ts(io, P)],
                                rhs=wos_sb[:, h, ot * 512 : ot * 512 + ow],
                                start=(h == 0),
                                stop=(h == HPC - 1),
                            )
                        nc.any.tensor_copy(o_sb[:, ot * 512 : ot * 512 + ow], pso)
                    nc.sync.dma_start(out_d[ts(io, P), :], o_sb)

    _split_excess_waits(nc)
    return nc


_NC = None


def _get_nc():
    global _NC
    if _NC is None:
        _NC = _build_module()
    return _NC


def _make_mask():
    p = np.arange(P)[:, None]
    f = np.arange(512)[None, :]
    m = np.empty((P, 4, 512), np.float32)
    for k in range(4):
        m[:, k, :] = (p <= f - P * k).astype(np.float32)
    return m


def kernel(x, wq, bq, wk, bk, wv, bv, wo, bo):
    x = np.asarray(x, np.float32)
    wq = np.asarray(wq, np.float32)
    bq = np.asarray(bq, np.float32)
    wk = np.asarray(wk, np.float32)
    bk = np.asarray(bk, np.float32)
    wv = np.asarray(wv, np.float32)
    bv = np.asarray(bv, np.float32)
    wo = np.asarray(wo, np.float32)
    bo = np.asarray(bo, np.float32)

    scale = 1.0 / np.sqrt(HD)
    mask = _make_mask()
    in_maps = []
    for core in range(8):
        b = core // 4
        h0 = (core % 4) * HPC
        heads = list(range(h0, h0 + HPC))

        wqk = np.empty((D, DQK), np.float32)
        bqk = np.empty((1, DQK), np.float32)
        for hl, hg in enumerate(heads):
            cs = slice(hg * HD, (hg + 1) * HD)
            wqk[:, hl * P : hl * P + HD] = wq[:, cs] * scale
            wqk[:, hl * P + HD : (hl + 1) * P] = wk[:, cs]
            bqk[0, hl * P : hl * P + HD] = bq[cs] * scale
            bqk[0, hl * P + HD : (hl + 1) * P] = bk[cs]

        vcols = slice(h0 * HD, (h0 + HPC) * HD)
        wos = (
            wo[vcols, :].reshape(HPC, HD, D).transpose(1, 0, 2).copy()
        )  # [HD, HPC, D]

        in_maps.append(
            {
                "xt": np.ascontiguousarray(x[b].T),
                "wqk": wqk,
                "bqk": bqk,
                "wv": np.ascontiguousarray(wv[:, vcols]),
                "bv": bv[None, vcols].copy(),
                "wos": wos,
                "mask": mask,
            }
        )

    res = run_bass_kernel_spmd(_get_nc(), in_maps, core_ids=list(range(8)))
    out = np.zeros((B, S, D), np.float32)
    for core in range(8):
        out[core // 4] += res.results[core]["out"]
    out += bo
    return out


# revision 4
# speedup vs baseline: 2.0438x; 2.0438x over previous
# Multi-head causal self-attention (B=2, S=2048, D=768, H=12) on 8 NeuronCores.
#
# Sharding: (batch, head-group) across cores. Core c handles batch c//4 and
# heads 3*(c%4) .. 3*(c%4)+2. Each core computes its heads' Q/K/V projections
# (column-sharded), the causal attention for those heads, and a row-sharded
# partial of the output projection. Host sums the 4 partials per batch + bo.
#
# All matmul operands are bf16 (fp32 matmuls run the PE array twice per
# instruction); accumulation stays fp32 in PSUM and softmax runs in fp32.
#
# Self-contained: hardcodes shapes; builds the Bass module once per process.

import sys

import ml_dtypes
import numpy as np

sys.path.insert(0, "/opt/trn_rl_repo")

import concourse.bass as bass  # noqa: E402
import concourse.mybir as mybir  # noqa: E402
import concourse.tile as tile  # noqa: E402
from concourse.bass import ts  # noqa: E402
from concourse.bass_utils import run_bass_kernel_spmd  # noqa: E402

F32 = mybir.dt.float32
BF16 = mybir.dt.bfloat16
AF = mybir.ActivationFunctionType
NPBF16 = ml_dtypes.bfloat16

B, S, D, H, HD = 2, 2048, 768, 12, 64
HPC = 3               # heads per core
DQK = 2 * HPC * HD    # 384: per-head-interleaved [Q_h | K_h] projection width
DV = HPC * HD         # 192
P = 128
IC = S // 512         # 4 query chunks of 512
KC = D // P           # 6 contraction chunks
NIO = S // P          # 16 token chunks of 128


def _split_excess_waits(nc, max_waits=1):
    # walrus in this env rejects instructions carrying more than ~1-2
    # sync-waits. Move excess waits onto preceding same-engine nops
    # (sequencer executes the nop's wait, then the instruction's).
    n_split = 0
    for func in nc.m.functions:
        for blk in func.blocks:
            insts = blk.instructions
            out = []
            changed = False
            for inst in insts:
                si = inst.sync_info
                waits = list(si.on_wait) if si and si.on_wait else []
                if len(waits) > max_waits:
                    changed = True
                    for j, w in enumerate(waits[:-max_waits]):
                        out.append(
                            mybir.InstNoOp(
                                name=f"{inst.name}-wsplit{j}",
                                engine=inst.engine,
                                ins=[],
                                outs=[],
                                sync_info=mybir.SyncInfo(
                                    on_wait=[w], on_update=[]
                                ),
                            )
                        )
                        n_split += 1
                    inst.sync_info = mybir.SyncInfo(
                        on_wait=waits[-max_waits:],
                        on_update=list(si.on_update) if si.on_update else [],
                    )
                out.append(inst)
            if changed:
                blk.instructions = out
    return n_split


def _build_module():
    nc = bass.Bass()
    xt_d = nc.dram_tensor("xt", [D, S], BF16, kind="ExternalInput")
    wqk_d = nc.dram_tensor("wqk", [D, DQK], BF16, kind="ExternalInput")
    bqk_d = nc.dram_tensor("bqk", [1, DQK], BF16, kind="ExternalInput")
    wv_d = nc.dram_tensor("wv", [D, DV], BF16, kind="ExternalInput")
    bv_d = nc.dram_tensor("bv", [1, DV], BF16, kind="ExternalInput")
    wos_d = nc.dram_tensor("wos", [HD, HPC, D], BF16, kind="ExternalInput")
    mask_d = nc.dram_tensor("mask", [P, 4, 512], BF16, kind="ExternalInput")
    out_d = nc.dram_tensor("out", [S, D], F32, kind="ExternalOutput")

    with tile.TileContext(nc) as tc:
        with (
            tc.tile_pool(name="const", bufs=1) as cp,
            tc.tile_pool(name="xtp", bufs=2) as xtp,
            tc.tile_pool(name="exp", bufs=3) as exp_p,
            tc.tile_pool(name="small", bufs=4) as sp,
            tc.tile_pool(name="outp", bufs=2) as op,
            tc.tile_pool(name="proj", bufs=2, space="PSUM") as proj_p,
            tc.tile_pool(name="scps", bufs=2, space="PSUM") as sc_p,
            tc.tile_pool(name="avps", bufs=2, space="PSUM") as av_p,
        ):
            # ---- resident SBUF tensors ----
            wqk_sb = cp.tile([P, KC, DQK], BF16)
            nc.sync.dma_start(wqk_sb, wqk_d.rearrange("(kc p) d -> p kc d", p=P))
            wv_sb = cp.tile([P, KC, DV], BF16)
            nc.sync.dma_start(wv_sb, wv_d.rearrange("(kc p) d -> p kc d", p=P))
            wos_sb = cp.tile([HD, HPC, D], BF16)
            nc.sync.dma_start(wos_sb, wos_d[:])
            bqk_sb = cp.tile([1, DQK], BF16)
            nc.sync.dma_start(bqk_sb, bqk_d[:])
            bv_sb = cp.tile([1, DV], BF16)
            nc.sync.dma_start(bv_sb, bv_d[:])
            mask_sb = cp.tile([P, 4, 512], BF16)
            nc.sync.dma_start(mask_sb, mask_d[:])
            ones_sb = cp.tile([1, 512], BF16)
            nc.gpsimd.memset(ones_sb, 1.0)
            ones_f = cp.tile([1, HD], F32)
            nc.gpsimd.memset(ones_f, 1.0)

            qT = cp.tile([HD, HPC, S], BF16)      # per-head Q^T  [d, h, i]
            klo = cp.tile([HD, HPC, S], BF16)     # per-head K^T  [d, h, j]
            # V plus a ones column (col HD) for the softmax denominator
            v1 = cp.tile([P, NIO, HPC, HD + 1], BF16)
            nc.gpsimd.memset(v1, 1.0)
            ctxT = cp.tile([HD, HPC, S], BF16)    # normalized ctx^T [d, h, i]

            for ic in range(IC):
                isl = ts(ic, 512)
                xt_t = xtp.tile([P, KC, 512], BF16, tag="xt")
                nc.sync.dma_start(
                    xt_t, xt_d.rearrange("(kc p) t -> p kc t", p=P)[:, :, isl]
                )

                # ---- QK projection: chunk h of psum = [Q_h | K_h] ----
                for h in range(HPC):
                    ps = proj_p.tile([P, 512], F32, tag="proj")
                    for kc in range(KC):
                        nc.tensor.matmul(
                            ps,
                            lhsT=wqk_sb[:, kc, ts(h, P)],
                            rhs=xt_t[:, kc, :],
                            start=(kc == 0),
                            stop=False,
                        )
                    nc.tensor.matmul(
                        ps,
                        lhsT=bqk_sb[:, ts(h, P)],
                        rhs=ones_sb[:, :],
                        start=False,
                        stop=True,
                    )
                    nc.vector.tensor_copy(qT[:, h, isl], ps[0:HD, :])
                    nc.vector.tensor_copy(klo[:, h, isl], ps[HD:P, :])

                # ---- V projection (natural layout, tokens on partitions) ----
                for io4 in range(4):
                    io = ic * 4 + io4
                    ps = proj_p.tile([P, 512], F32, tag="proj")
                    psv = ps[:, :DV]
                    for kc in range(KC):
                        nc.tensor.matmul(
                            psv,
                            lhsT=xt_t[:, kc, ts(io4, P)],
                            rhs=wv_sb[:, kc, :],
                            start=(kc == 0),
                            stop=False,
                        )
                    nc.tensor.matmul(
                        psv,
                        lhsT=ones_sb[:, 0:P],
                        rhs=bv_sb[:, :],
                        start=False,
                        stop=True,
                    )
                    nc.vector.tensor_copy(
                        v1[:, io, :, 0:HD],
                        psv.rearrange("p (h e) -> p h e", e=HD),
                    )

                # ---- attention for queries in this chunk ----
                n_j = 4 * ic + 4
                for h in range(HPC):
                    avp = av_p.tile([HD + 1, 512], F32, tag="av")
                    prev = None
                    for jb in range(0, n_j, 2):
                        sc = sc_p.tile([P, 2, 512], F32, tag="sc")
                        for k in range(2):
                            jc = jb + k
                            nc.tensor.matmul(
                                sc[:, k, :],
                                lhsT=klo[:, h, ts(jc, P)],
                                rhs=qT[:, h, isl],
                                start=True,
                                stop=True,
                            )
                        ex = exp_p.tile([P, 2, 512], BF16, tag="ex")
                        nc.scalar.activation(ex, sc, AF.Exp)
                        for k in range(2):
                            koff = (jb + k) - 4 * ic
                            if koff >= 0:
                                nc.vector.tensor_mul(
                                    ex[:, k, :], ex[:, k, :], mask_sb[:, koff, :]
                                )
                        if prev is not None:
                            pex, pjb = prev
                            for k in range(2):
                                jc = pjb + k
                                nc.tensor.matmul(
                                    avp,
                                    lhsT=v1[:, jc, h, :],
                                    rhs=pex[:, k, :],
                                    start=(jc == 0),
                                    stop=(jc == n_j - 1),
                                )
                        prev = (ex, jb)
                    pex, pjb = prev
                    for k in range(2):
                        jc = pjb + k
                        nc.tensor.matmul(
                            avp,
                            lhsT=v1[:, jc, h, :],
                            rhs=pex[:, k, :],
                            start=(jc == 0),
                            stop=(jc == n_j - 1),
                        )
                    # normalize: ctxT = avp[0:HD] * (1/Z); Z broadcast via
                    # a K=1 ones-matmul (PE is the partition broadcaster)
                    z_sb = sp.tile([1, 512], F32, tag="z")
                    nc.scalar.activation(z_sb, avp[HD : HD + 1, :], AF.Copy)
                    rb_ps = proj_p.tile([P, 512], F32, tag="proj")
                    nc.tensor.matmul(
                        rb_ps[0:HD, :],
                        lhsT=ones_f[0:1, :],
                        rhs=z_sb[:, :],
                        start=True,
                        stop=True,
                    )
                    rb_sb = sp.tile([HD, 512], F32, tag="rb")
                    nc.vector.reciprocal(rb_sb, rb_ps[0:HD, :])
                    nc.vector.tensor_tensor(
                        ctxT[:, h, isl],
                        avp[0:HD, :],
                        rb_sb,
                        mybir.AluOpType.mult,
                    )

                # ---- output projection for this chunk's tokens ----
                for io4 in range(4):
                    io = ic * 4 + io4
                    o_sb = op.tile([P, D], F32, tag="osb")
                    for ot, ow in ((0, 512), (1, 256)):
                        ps = proj_p.tile([P, 512], F32, tag="proj")
                        pso = ps[:, :ow]
                        for h in range(HPC):
                            nc.tensor.matmul(
                                pso,
                                lhsT=ctxT[:, h, ts(io, P)],
                                rhs=wos_sb[:, h, ot * 512 : ot * 512 + ow],
                                start=(h == 0),
                                stop=(h == HPC - 1),
                            )
                        nc.any.tensor_copy(o_sb[:, ot * 512 : ot * 512 + ow], pso)
                    nc.sync.dma_start(out_d[ts(io, P), :], o_sb)

    _split_excess_waits(nc)
    return nc


_NC = None


def _get_nc():
    global _NC
    if _NC is None:
        _NC = _build_module()
    return _NC


def _make_mask():
    p = np.arange(P)[:, None]
    f = np.arange(512)[None, :]
    m = np.empty((P, 4, 512), np.float32)
    for k in range(4):
        m[:, k, :] = (p <= f - P * k).astype(np.float32)
    return m.astype(NPBF16)


def _build_in_maps(x, wq, bq, wk, bk, wv, bv, wo):
    scale = 1.0 / np.sqrt(HD)
    mask = _make_mask()
    in_maps = []
    for core in range(8):
        b = core // 4
        h0 = (core % 4) * HPC
        heads = list(range(h0, h0 + HPC))

        wqk = np.empty((D, DQK), np.float32)
        bqk = np.empty((1, DQK), np.float32)
        for hl, hg in enumerate(heads):
            cs = slice(hg * HD, (hg + 1) * HD)
            wqk[:, hl * P : hl * P + HD] = wq[:, cs] * scale
            wqk[:, hl * P + HD : (hl + 1) * P] = wk[:, cs]
            bqk[0, hl * P : hl * P + HD] = bq[cs] * scale
            bqk[0, hl * P + HD : (hl + 1) * P] = bk[cs]

        vcols = slice(h0 * HD, (h0 + HPC) * HD)
        wos = (
            wo[vcols, :].reshape(HPC, HD, D).transpose(1, 0, 2)
        )  # [HD, HPC, D]

        in_maps.append(
            {
                "xt": np.ascontiguousarray(x[b].T).astype(NPBF16),
                "wqk": wqk.astype(NPBF16),
                "bqk": bqk.astype(NPBF16),
                "wv": np.ascontiguousarray(wv[:, vcols]).astype(NPBF16),
                "bv": bv[None, vcols].astype(NPBF16),
                "wos": np.ascontiguousarray(wos).astype(NPBF16),
                "mask": mask,
            }
        )
    return in_maps


def kernel(x, wq, bq, wk, bk, wv, bv, wo, bo):
    x = np.asarray(x, np.float32)
    wq = np.asarray(wq, np.float32)
    bq = np.asarray(bq, np.float32)
    wk = np.asarray(wk, np.float32)
    bk = np.asarray(bk, np.float32)
    wv = np.asarray(wv, np.float32)
    bv = np.asarray(bv, np.float32)
    wo = np.asarray(wo, np.float32)
    bo = np.asarray(bo, np.float32)

    in_maps = _build_in_maps(x, wq, bq, wk, bk, wv, bv, wo)
    res = run_bass_kernel_spmd(_get_nc(), in_maps, core_ids=list(range(8)))
    out = np.zeros((B, S, D), np.float32)
    for core in range(8):
        out[core // 4] += res.results[core]["out"]
    out += bo
    return out
